# revision 22
# baseline (speedup 1.0000x reference)
"""Edge-parallel GNN u_mul_v kernel for Trainium2 (8 NeuronCores).

z[e, :] = h[src[e], :] * h[dst[e], :]

Shard edges across 8 cores (100K each); h (12.8MB) replicated in HBM as the
gather table, addressed as two int16-indexable tables (h[:32768], h[32768:]).
Rows are fetched with the SWDGE InstDMAGatherAnt (nc.gpsimd.dma_gather);
measured desc-generation runs ~2.2ns/descriptor aggregate (4 queues), so
descriptor COUNT is the wall.

Src-side token pairing cuts descriptors ~19%: edges are sorted by src, and
two edges whose src rows fall in the same 512B pair-token (rows 2t, 2t+1)
share ONE elem=128 descriptor. The pair's members land in z slots (p, 2m)
and (p, 2m+1) -- same partition, adjacent slots -- so the multiply reads the
shared gather slot through static strided views with per-tile parity offsets
(pairs bucketed by (parity1<=parity2) pattern: 3 patterns x 4 table-classes).
Unpairable edges go through the original per-edge path (elem=64).

z is stored fp16 (gate 2e-2; fp16 product error ~1e-3); host upcasts.
All index tiles are preloaded; gathers round-robin the 4 SWDGE queues.
"""

import numpy as np

N_NODES = 50000
N_EDGES = 800000
D = 64
N_CORES = 8
E_PER_CORE = N_EDGES // N_CORES  # 100000
L = 32768  # int16-addressable rows per gather table
NP = 2048  # pairs per pair-tile (src gather descs; dst gather = 2*NP)
NS = 4096  # edges per single-tile

_cached = {}


def _build(tiles):
    """tiles: list of ("p", s_hi, d_hi, offA, offB, npairs) or
    ("s", s_hi, d_hi, ni)."""
    import concourse.bass as bass
    import concourse.tile as tile
    from concourse import bacc, mybir

    T = len(tiles)
    E_DEV = sum((2 * t[5]) if t[0] == "p" else t[3] for t in tiles)
    nc = bacc.Bacc(
        "TRN2",
        target_bir_lowering=False,
        debug=False,
        num_devices=N_CORES,
        num_swdge_queues=4,
    )
    h_ap = nc.dram_tensor("h", [N_NODES, D], mybir.dt.float32, kind="ExternalInput").ap()
    # per-tile idx tensors are packed into one [T, 128, NS//16] input;
    # pair tiles use [:, :NP//16] for src and [:, :2*NP//16] for dst.
    si_ap = nc.dram_tensor(
        "src_idx", [T, 128, NS // 16], mybir.dt.int16, kind="ExternalInput"
    ).ap()
    di_ap = nc.dram_tensor(
        "dst_idx", [T, 128, NS // 16], mybir.dt.int16, kind="ExternalInput"
    ).ap()
    z_ap = nc.dram_tensor("z", [E_DEV, D], mybir.dt.float16, kind="ExternalOutput").ap()

    tab = {0: h_ap[0:L, :], 1: h_ap[L:N_NODES, :]}
    # pair tables: parity par covers base rows par, par+2, ... so a desc at
    # idx q delivers rows [par+2q, par+2q+1] -- any adjacent row window.
    def _ptab(base, rows, par):
        n2 = (rows - par) // 2
        return h_ap[base + par : base + par + 2 * n2, :].rearrange(
            "(n two) d -> n (two d)", two=2
        )

    ptab = {
        (0, 0): _ptab(0, L, 0),
        (0, 1): _ptab(0, L, 1),
        (1, 0): _ptab(L, N_NODES - L, 0),
        (1, 1): _ptab(L, N_NODES - L, 1),
    }

    with tile.TileContext(nc) as tc:
        with (
            tc.tile_pool(name="ix", bufs=1) as ixp,
            tc.tile_pool(name="ga", bufs=4) as gap,
            tc.tile_pool(name="gb", bufs=4) as gbp,
            tc.tile_pool(name="zz", bufs=3) as zp,
        ):
            sixs, dixs = [], []
            for t, tl in enumerate(tiles):
                ns_ = (tl[5] // 16) if tl[0] == "p" else (tl[3] // 16)
                nd_ = (2 * tl[5] // 16) if tl[0] == "p" else (tl[3] // 16)
                six = ixp.tile([128, ns_], mybir.dt.int16, tag=f"six{t}")
                nc.sync.dma_start(six[:], si_ap[t][:, :ns_])
                sixs.append(six)
                dix = ixp.tile([128, nd_], mybir.dt.int16, tag=f"dix{t}")
                nc.sync.dma_start(dix[:], di_ap[t][:, :nd_])
                dixs.append(dix)
            # All gathers are issued as uniform <=2048-desc chunks on a
            # round-robin queue counter: mixed gather sizes caused
            # head-of-line blocking in the in-order Pool dispatch stream.
            qc = [0]

            def gath(out_tile, in_tab, idx_tile, total, elem):
                s = 0
                while s < total:
                    n = min(2048, total - s)
                    nc.gpsimd.dma_gather(
                        out_ap=out_tile[:, s // 128 : (s + n) // 128],
                        in_ap=in_tab,
                        idxs_ap=idx_tile[:, s // 16 : (s + n) // 16],
                        num_idxs=n, num_idxs_reg=n, elem_size=elem,
                        single_packet=False, queue_num=qc[0] % 4,
                    )
                    qc[0] += 1
                    s += n

            base = 0
            for t, tl in enumerate(tiles):
                if tl[0] == "p":
                    _, s_hi, d_hi, par, delta, npr = tl
                    offA, offB = 0, delta
                    m = npr // 128
                    ga = gap.tile([128, m, 2 * D], mybir.dt.float32, tag="gap")
                    gath(ga, ptab[(s_hi, par)], sixs[t], npr, 2 * D)
                    gb = gbp.tile([128, 2 * m, D], mybir.dt.float32, tag="gbp")
                    gath(gb, tab[d_hi], dixs[t], 2 * npr, D)
                    zt = zp.tile([128, 2 * m, D], mybir.dt.float16, tag="zp")
                    gbv = gb[:].rearrange("p (m two) d -> p m two d", two=2)
                    ztv = zt[:].rearrange("p (m two) d -> p m two d", two=2)
                    nc.vector.tensor_mul(
                        ztv[:, :, 0, :], ga[:, :, offA * D : (offA + 1) * D],
                        gbv[:, :, 0, :],
                    )
                    nc.vector.tensor_mul(
                        ztv[:, :, 1, :], ga[:, :, offB * D : (offB + 1) * D],
                        gbv[:, :, 1, :],
                    )
                    ni = 2 * npr
                else:
                    _, s_hi, d_hi, ni = tl
                    g = ni // 128
                    ga = gap.tile([128, g, D], mybir.dt.float32, tag="gas")
                    gath(ga, tab[s_hi], sixs[t], ni, D)
                    gb = gbp.tile([128, g, D], mybir.dt.float32, tag="gbs")
                    gath(gb, tab[d_hi], dixs[t], ni, D)
                    zt = zp.tile([128, g, D], mybir.dt.float16, tag="zs")
                    nc.vector.tensor_mul(zt[:], ga[:], gb[:])
                z_view = z_ap[base : base + ni, :].rearrange(
                    "(p gd) d -> p (gd d)", p=128
                )
                nc.sync.dma_start(z_view, zt[:])
                base += ni
    nc.compile()
    return nc


def _wrap16(a):
    w = np.asarray(a, np.int16).reshape(-1, 16).T
    return np.ascontiguousarray(np.tile(w, (8, 1)))


def _pair_class(s_loc, eids, nrows):
    """Edges of one (core, table-class), s_loc sorted ascending. Greedy
    adjacent matching: consecutive edges whose rows differ by <=1 share a
    [rA, rA+1] window descriptor (needs rA+1 < nrows).
    Returns pair_a, pair_b (positions), singles positions."""
    n = len(s_loc)
    pa, pb, sg = [], [], []
    i = 0
    while i < n:
        if (
            i + 1 < n
            and s_loc[i + 1] - s_loc[i] <= 1
            and s_loc[i] + 1 < nrows
        ):
            pa.append(i)
            pb.append(i + 1)
            i += 2
        else:
            sg.append(i)
            i += 1
    return (
        np.array(pa, np.int64),
        np.array(pb, np.int64),
        np.array(sg, np.int64),
    )


def _prepare(src, dst):
    src = np.asarray(src).astype(np.int64)
    dst = np.asarray(dst).astype(np.int64)
    # Pair GLOBALLY over all 800K edges (8x the per-core row occupancy ->
    # almost every edge finds a <=1-row partner), then deal each bucket
    # evenly across the 8 cores (sharding is ours to choose; dev_orig
    # handles the unshard). 16 pair buckets (class, parity, delta) + 4
    # single buckets (class).
    cls_all = (src >= L).astype(np.int64) * 2 + (dst >= L).astype(np.int64)
    pair_b = [[None] * 16 for _ in range(N_CORES)]  # (pairs[n,2], pidx[n])
    sing_b = [[None] * 4 for _ in range(N_CORES)]  # eids[n]
    for k in range(4):
        e = np.where(cls_all == k)[0]
        s_hi = k >> 1
        nrows = L if s_hi == 0 else N_NODES - L
        sl = src[e] - s_hi * L
        o = np.argsort(sl, kind="stable")
        e, sl = e[o], sl[o]
        pa, pb, sg = _pair_class(sl, e, nrows)
        rA = sl[pa] if len(pa) else np.zeros(0, np.int64)
        delta = (sl[pb] - rA) if len(pa) else np.zeros(0, np.int64)
        par = rA & 1
        pidx = rA >> 1
        pairs = (
            np.stack([e[pa], e[pb]], axis=1)
            if len(pa)
            else np.zeros((0, 2), np.int64)
        )
        for pr in range(2):
            for dl in range(2):
                m = (par == pr) & (delta == dl)
                for c, (pc, ic) in enumerate(
                    zip(np.array_split(pairs[m], N_CORES),
                        np.array_split(pidx[m], N_CORES))
                ):
                    pair_b[c][k * 4 + pr * 2 + dl] = (pc, ic)
        for c, ec in enumerate(np.array_split(e[sg], N_CORES)):
            sing_b[c][k] = ec
    pcaps = [
        -(-max(len(pair_b[c][j][0]) for c in range(N_CORES)) // 128) * 128
        for j in range(16)
    ]
    scaps = [
        -(-max(len(sing_b[c][k]) for c in range(N_CORES)) // 128) * 128
        for k in range(4)
    ]
    # tiles: full pair tiles and full single tiles round-robin, tails last.
    tiles = []
    meta = []  # ("p", j, start) / ("s", k, start)
    pcur = [0] * 16
    scur = [0] * 4
    emitted = True
    while emitted:
        emitted = False
        for j in range(16):
            if pcaps[j] - pcur[j] >= NP:
                k, pr, dl = j // 4, (j % 4) // 2, j % 2
                tiles.append(("p", k >> 1, k & 1, pr, dl, NP))
                meta.append(("p", j, pcur[j]))
                pcur[j] += NP
                emitted = True
        for k in range(4):
            if scaps[k] - scur[k] >= NS:
                tiles.append(("s", k >> 1, k & 1, NS))
                meta.append(("s", k, scur[k]))
                scur[k] += NS
                emitted = True
    for j in range(16):
        rem = pcaps[j] - pcur[j]
        if rem > 0:
            k, pr, dl = j // 4, (j % 4) // 2, j % 2
            tiles.append(("p", k >> 1, k & 1, pr, dl, rem))
            meta.append(("p", j, pcur[j]))
            pcur[j] += rem
    for k in range(4):
        rem = scaps[k] - scur[k]
        if rem > 0:
            tiles.append(("s", k >> 1, k & 1, rem))
            meta.append(("s", k, scur[k]))
            scur[k] += rem
    T = len(tiles)
    tile_ni = [(2 * t[5]) if t[0] == "p" else t[3] for t in tiles]
    E_DEV = sum(tile_ni)
    tile_bases = np.cumsum([0] + tile_ni)

    in_maps = []
    dev_orig = np.empty((N_CORES, E_DEV), np.int64)
    for c in range(N_CORES):
        si = np.zeros((T, 128, NS // 16), np.int16)
        di = np.zeros((T, 128, NS // 16), np.int16)
        for t, (tl, (kind, jk, start)) in enumerate(zip(tiles, meta)):
            zb = tile_bases[t]
            if kind == "p":
                npr = tl[5]
                pairs_all, tok_all = pair_b[c][jk]
                seg_p = np.full((npr, 2), -1, np.int64)
                seg_t = np.zeros(npr, np.int64)
                avail = max(0, min(len(tok_all) - start, npr))
                if avail > 0:
                    seg_p[:avail] = pairs_all[start : start + avail]
                    seg_t[:avail] = tok_all[start : start + avail]
                si[t, :, : npr // 16] = _wrap16(seg_t)
                # dst positions: t2 -> (p=t2%128, slot=t2//128); edge at
                # (p, 2q+w) = pair[q*128+p][w]
                m = npr // 128
                ebpj = np.empty((128, 2 * m), np.int64)
                pr = seg_p.reshape(m, 128, 2)
                for w in range(2):
                    ebpj[:, w::2] = pr[:, :, w].T
                dseq = ebpj.T.reshape(-1)  # position order
                d_hi = tl[2]
                dloc = np.where(dseq >= 0, dst[np.maximum(dseq, 0)] - d_hi * L, 0)
                di[t, :, : 2 * npr // 16] = _wrap16(dloc)
                # device row r = p*(2m) + j ; edge = pair[(j//2)*128+p][j%2]
                rr = np.arange(2 * npr)
                p_, j_ = rr // (2 * m), rr % (2 * m)
                dev_orig[c, zb : zb + 2 * npr] = seg_p[(j_ // 2) * 128 + p_, j_ % 2]
            else:
                ni = tl[3]
                se = sing_b[c][jk]
                seg = np.full(ni, -1, np.int64)
                avail = max(0, min(len(se) - start, ni))
                if avail > 0:
                    seg[:avail] = se[start : start + avail]
                s_hi, d_hi = tl[1], tl[2]
                sloc = np.where(seg >= 0, src[np.maximum(seg, 0)] - s_hi * L, 0)
                dloc = np.where(seg >= 0, dst[np.maximum(seg, 0)] - d_hi * L, 0)
                si[t, :, : ni // 16] = _wrap16(sloc)
                di[t, :, : ni // 16] = _wrap16(dloc)
                g = ni // 128
                tmap = np.arange(ni).reshape(g, 128).T.reshape(-1)
                dev_orig[c, zb : zb + ni] = seg[tmap]
        in_maps.append({"si": si, "di": di})
    return tiles, in_maps, dev_orig


def _get_nc(tiles):
    key = tuple(tiles)
    if key not in _cached:
        _cached[key] = _build(list(key))
    return _cached[key]


def _make_in_maps(h, src, dst):
    tiles, idx_maps, dev_orig = _prepare(src, dst)
    h32 = np.ascontiguousarray(h, dtype=np.float32)
    in_maps = [
        {"h": h32, "src_idx": m["si"], "dst_idx": m["di"]} for m in idx_maps
    ]
    return tiles, in_maps, dev_orig


def kernel(h, src, dst):
    from concourse import bass_utils

    tiles, in_maps, dev_orig = _make_in_maps(h, src, dst)
    nc = _get_nc(tiles)
    res = bass_utils.run_bass_kernel_spmd(nc, in_maps, list(range(N_CORES)))
    out = np.empty((N_EDGES, D), np.float32)
    for c in range(N_CORES):
        zc = res.results[c]["z"]
        valid = dev_orig[c] >= 0
        out[dev_orig[c][valid]] = zc[valid].astype(np.float32)
    return out
